# revision 11
# baseline (speedup 1.0000x reference)
"""ExpLog Dice loss kernel for Trainium2 (8 NeuronCores, SPMD data-parallel).

Math
----
reference computes, for cls_score [N, C] and integer labels [N]:
    log_probs = log_softmax(cls_score, axis=1)
    ni_c  = logsumexp_{n: label==c} log_probs[n, c]
    npr_c = logsumexp_n           log_probs[n, c]
    counts_c = #{n: label==c}
    ... tiny C-length final loss.

Since cls_score ~ N(0,1), exp(x) never overflows fp32, so logsumexps become
plain sums of probabilities:
    S_c = sum_n exp(x[n,c]) / D_n        (npr_c = log S_c)
    T_c = sum_{n:label=c} exp(x[n,c])/D_n (ni_c = log T_c)
    D_n = sum_c exp(x[n,c])

v3 design (per core, N/8 = 131072 points = 128 partitions x 1024 pages)
----------------------------------------------------------------------
Inputs ship as bf16 strips, one per tile of S pages: [128, S + 32*S]
  cols [0,S)       g block: gathered true-class score per page
  cols [S, 33*S)   x block: the 32 class scores per page, page-major
Device pipeline per tile (ACT is the wall: 1 elem/lane/cycle exp):
  ACT : e = exp(strip)                 (one instr, 33*S elems/partition)
  DVE : den[:, off:off+S] = reduce_add over class axis of x block (bf16 2x)
  DVE : rec = 1/den   (batched over ~3 tiles to amortize instr overhead)
  GPS : recb = bf16(rec) (batched), w = e_g * recb  (per tile)
  PE  : per 16-page group, lhsT=recb cols, rhs=e x-block cols ->
        accumulate [16, 512] PSUM whose diagonal 16x(16x32) blocks are
        the per-class partial sums of e/D.
  out : w [128,1024] bf16 (two chunks, mid-stream + end), PSUM [16,512] f32.
Host: bincount(label, weights=w) -> T_c, counts; diag-sum of PSUM -> S_c;
tiny C-length final loss in float64.

bf16 input rationale: memory-regime problem; the kernel's internal
precision choice. Per-element rel error ~2^-9 on x => <=1.1% on exp;
class sums average 4096+ points per core so final loss error ~1e-4,
far inside the 2e-2 gate.
"""

import sys

for _p in ("/opt/trn_rl_repo", "/root/.axon_site/_ro/trn_rl_repo"):
    if _p not in sys.path:
        sys.path.insert(0, _p)

from contextlib import ExitStack

import numpy as np
import ml_dtypes

import concourse.bass as bass
from concourse import mybir, tile

# ---------------- problem constants (hardcoded per contract) ----------------
N_TOTAL = 1048576
C = 32
NCORES = 8
N_CORE = N_TOTAL // NCORES  # 131072
P = 128
PAGES = N_CORE // P         # 1024 pages per partition
GM = 16                     # pages per matmul group == PSUM M dim

# tile sizes in pages: small first (prime ACT early), big middle, small tail.
# All multiples of GM=16 so every matmul hits the identical [16, 512] PSUM
# region (one accumulation group; mismatched sub-regions hang the PE).
# Fewer tiles amortize the ~390ns fixed cost per bf16 ACTIVATE.
SPLITS = [16, 32, 64, 128, 128, 128, 128, 128, 128, 96, 32, 16]
assert sum(SPLITS) == PAGES, sum(SPLITS)
OFFS = [0]
for s in SPLITS:
    OFFS.append(OFFS[-1] + s)
# recip/cast batches: process den->rec->recb in groups of tiles
# recip/cast batch ends: batched early (DVE has slack), per-tile late so
# the PE matmul burst for big late tiles starts right after that tile's
# own tree instead of waiting for its batch-mate.
RBATCH_END = [1, 3, 5, 7, 8, 9, 10, 11]
W_SPLIT_TILE = 9                # after this tile's w-mult, DMA w[:, :cum]

GAMMA = 0.3
LOSS_WEIGHT = 1.0
LG2 = 0.6931471805599453

BF16 = ml_dtypes.bfloat16


# ---------------- kernel builder (v3) --------------------------------------
def build_nc_v3():
    f32 = mybir.dt.float32
    bf16 = mybir.dt.bfloat16
    nc = bass.Bass()
    strips = [
        nc.dram_tensor(f"s{t}", [P, 33 * s], bf16, kind="ExternalInput")
        for t, s in enumerate(SPLITS)
    ]
    out_d = nc.dram_tensor("out", [GM, GM * C], f32, kind="ExternalOutput")
    w_d = nc.dram_tensor("wout", [P, PAGES], bf16, kind="ExternalOutput")

    with tile.TileContext(nc) as tc, ExitStack() as ctx:
        pool = ctx.enter_context(tc.tile_pool(name="work", bufs=6))
        tpool = ctx.enter_context(tc.tile_pool(name="tree", bufs=2))
        once = ctx.enter_context(tc.tile_pool(name="once", bufs=1))
        psum = ctx.enter_context(
            tc.tile_pool(name="psum", bufs=1, space=bass.MemorySpace.PSUM)
        )
        ps_s = psum.tile([GM, GM * C], f32)

        den_all = once.tile([P, PAGES], f32)
        rec_all = once.tile([P, PAGES], f32)
        recb_all = once.tile([P, PAGES], bf16)
        w_all = once.tile([P, PAGES], bf16)

        n_mm = sum((s + GM - 1) // GM for s in SPLITS)
        mm_done = 0
        rb = 0  # current recip batch index
        cur = []  # (off, S, ex) of tiles in the open batch
        for t, S in enumerate(SPLITS):
            off = OFFS[t]
            strip = pool.tile([P, 33 * S], bf16, tag="x")
            nc.sync.dma_start(strip[:], strips[t][:])

            ex = pool.tile([P, 33 * S], bf16, tag="e")
            nc.scalar.activation(ex[:], strip[:], mybir.ActivationFunctionType.Exp)
            x3 = ex[:, S:].rearrange("p (s n) -> p s n", n=C)

            # D-reduce as a pairwise tensor_tensor tree: tensor_reduce only
            # has a 1x DVE uop, but bf16 packed tensor_tensor runs 2x.
            src = x3
            width = C
            while width > 2:
                half = width // 2
                nxt = tpool.tile([P, S * half], bf16, tag=f"t{half}")
                n3 = nxt[:].rearrange("p (s n) -> p s n", n=half)
                nc.vector.tensor_tensor(
                    n3,
                    src[:, :, 0:half],
                    src[:, :, half:width],
                    mybir.AluOpType.add,
                )
                src = n3
                width = half
            nc.vector.tensor_reduce(
                den_all[:, off : off + S],
                src,
                axis=mybir.AxisListType.X,
                op=mybir.AluOpType.add,
            )
            cur.append((off, S, ex))

            if t == RBATCH_END[rb]:
                b0 = cur[0][0]
                b1 = OFFS[t + 1]
                nc.vector.reciprocal_approx_fast(
                    rec_all[:, b0:b1], den_all[:, b0:b1]
                )
                with nc.allow_low_precision(reason="bf16 lhsT for PE matmul"):
                    nc.vector.tensor_copy(
                        recb_all[:, b0:b1], rec_all[:, b0:b1]
                    )
                # consumers of recb emitted only after the cast (same-engine
                # queues execute in emission order; cross-engine deps are
                # recorded against emission-time writers)
                for uoff, uS, uex in cur:
                    with nc.allow_low_precision(reason="bf16 w output"):
                        nc.gpsimd.tensor_tensor(
                            w_all[:, uoff : uoff + uS],
                            uex[:, 0:uS],
                            recb_all[:, uoff : uoff + uS],
                            mybir.AluOpType.mult,
                        )
                    for gs in range(0, uS, GM):
                        gl = min(GM, uS - gs)
                        nc.tensor.matmul(
                            ps_s[0:gl, 0 : gl * C],
                            recb_all[:, uoff + gs : uoff + gs + gl],
                            uex[:, uS + gs * C : uS + (gs + gl) * C],
                            start=(mm_done == 0),
                            stop=(mm_done == n_mm - 1),
                        )
                        mm_done += 1
                cur = []
                rb += 1

            if t == W_SPLIT_TILE:
                nc.sync.dma_start(w_d[:, 0 : OFFS[t + 1]], w_all[:, 0 : OFFS[t + 1]])

        nc.sync.dma_start(
            w_d[:, OFFS[W_SPLIT_TILE + 1] :], w_all[:, OFFS[W_SPLIT_TILE + 1] :]
        )
        stage = pool.tile([GM, GM * C], f32, tag="stage")
        nc.scalar.copy(stage[:], ps_s[:])
        nc.sync.dma_start(out_d[:], stage[:])
    return nc


def _finalize_for_hw(nc):
    """Lowerings required by the walrus compile path (not CoreSim)."""
    _split_multi_waits(nc)
    mybir.codegen_inst_isa_subclasses(nc)
    return nc


def _split_multi_waits(nc):
    """Walrus encodes exactly one sync-wait per ISA instruction; Tile can
    attach several. Hoist all-but-the-last wait onto single-wait NoOps
    inserted just before the instruction on the same engine (the sequencer
    executes them in order, so semantics are preserved)."""
    for fn in nc.m.functions:
        for blk in fn.blocks:
            new_list = []
            for ins in blk.instructions:
                si = ins.sync_info
                if si is not None and len(si.on_wait) > 1:
                    waits = list(si.on_wait)
                    for w in waits[:-1]:
                        nop = mybir.InstNoOp(
                            name=f"WS-{nc.next_id()}", ins=[], outs=[]
                        )
                        nop.engine = ins.engine
                        nop.sync_info = mybir.SyncInfo(on_wait=[w], on_update=[])
                        new_list.append(nop)
                    ins.sync_info = mybir.SyncInfo(
                        on_wait=[waits[-1]], on_update=list(si.on_update)
                    )
                new_list.append(ins)
            blk.instructions[:] = new_list


_NC_CACHE = {}


def get_nc():
    if "v3" not in _NC_CACHE:
        _NC_CACHE["v3"] = _finalize_for_hw(build_nc_v3())
    return _NC_CACHE["v3"]


# ---------------- host-side driver ------------------------------------------
def prep_in_maps(cls_score: np.ndarray, label: np.ndarray):
    cls_score = np.ascontiguousarray(cls_score, dtype=np.float32)
    lab = label.astype(np.int64)
    g = cls_score[np.arange(cls_score.shape[0]), lab]
    xb = cls_score.astype(BF16)
    gb = g.astype(BF16)
    in_maps = []
    for k in range(NCORES):
        base = k * N_CORE
        m = {}
        for t, S in enumerate(SPLITS):
            a = base + OFFS[t] * P
            b = a + S * P
            xt = xb[a:b].reshape(P, S * C)
            gt = gb[a:b].reshape(P, S)
            m[f"s{t}"] = np.ascontiguousarray(np.concatenate([gt, xt], axis=1))
        in_maps.append(m)
    return in_maps


def finalize(outs, label: np.ndarray):
    lab = label.astype(np.int64)
    acc = np.zeros((GM, GM * C), dtype=np.float64)
    w_parts = []
    for o in outs:
        acc += o["out"].astype(np.float64)
        w = o["wout"].astype(np.float64)  # [P, PAGES]
        w_parts.append(
            np.concatenate(
                [
                    w[:, OFFS[t] : OFFS[t + 1]].reshape(-1)
                    for t in range(len(SPLITS))
                ]
            )
        )
    blocks = acc.reshape(GM, GM, C)
    s_c = np.zeros(C, dtype=np.float64)
    for mrow in range(GM):
        s_c += blocks[mrow, mrow]

    w_all = np.concatenate(w_parts)
    t_c = np.bincount(lab, weights=w_all, minlength=C)
    counts = np.bincount(lab, minlength=C).astype(np.float64)
    present = counts > 0
    ni = np.log(np.maximum(t_c, 1e-300))
    npr = np.log(np.maximum(s_c, 1e-300))
    log_ngt = np.log(np.maximum(counts, 1.0))
    log_dice = LG2 + ni - np.logaddexp(log_ngt, npr)
    neg_log_dice = np.where(present, -log_dice, 1.0)
    losses = np.where(present, np.power(np.maximum(neg_log_dice, 0.0), GAMMA), 0.0)
    n_present = present.sum()
    return np.float32(LOSS_WEIGHT * losses.sum() / n_present)


def kernel(cls_score: np.ndarray, label: np.ndarray) -> np.ndarray:
    from concourse.bass_utils import run_bass_kernel_spmd

    cls_score = np.asarray(cls_score)
    label = np.asarray(label)
    assert cls_score.shape == (N_TOTAL, C), cls_score.shape
    nc = get_nc()
    in_maps = prep_in_maps(cls_score, label)
    res = run_bass_kernel_spmd(nc, in_maps, core_ids=list(range(NCORES)))
    return finalize(res.results, label)


if __name__ == "__main__":
    rng = np.random.default_rng(0)
    x = rng.standard_normal((N_TOTAL, C), dtype=np.float32)
    lab = rng.integers(0, C, N_TOTAL).astype(np.int32)
    print("loss:", kernel(x, lab))


# revision 12
# speedup vs baseline: 1.1170x; 1.1170x over previous
"""ExpLog Dice loss kernel for Trainium2 (8 NeuronCores, SPMD data-parallel).

Math
----
reference computes, for cls_score [N, C] and integer labels [N]:
    log_probs = log_softmax(cls_score, axis=1)
    ni_c  = logsumexp_{n: label==c} log_probs[n, c]
    npr_c = logsumexp_n           log_probs[n, c]
    counts_c = #{n: label==c}
    ... tiny C-length final loss.

Since cls_score ~ N(0,1), exp(x) never overflows fp32, so logsumexps become
plain sums of probabilities:
    S_c = sum_n exp(x[n,c]) / D_n        (npr_c = log S_c)
    T_c = sum_{n:label=c} exp(x[n,c])/D_n (ni_c = log T_c)
    D_n = sum_c exp(x[n,c])

v3 design (per core, N/8 = 131072 points = 128 partitions x 1024 pages)
----------------------------------------------------------------------
Inputs ship as bf16 strips, one per tile of S pages: [128, S + 32*S]
  cols [0,S)       g block: gathered true-class score per page
  cols [S, 33*S)   x block: the 32 class scores per page, page-major
Device pipeline per tile (ACT is the wall: 1 elem/lane/cycle exp):
  ACT : e = exp(strip)                 (one instr, 33*S elems/partition)
  DVE : den[:, off:off+S] = reduce_add over class axis of x block (bf16 2x)
  DVE : rec = 1/den   (batched over ~3 tiles to amortize instr overhead)
  GPS : recb = bf16(rec) (batched), w = e_g * recb  (per tile)
  PE  : per 16-page group, lhsT=recb cols, rhs=e x-block cols ->
        accumulate [16, 512] PSUM whose diagonal 16x(16x32) blocks are
        the per-class partial sums of e/D.
  out : w [128,1024] bf16 (two chunks, mid-stream + end), PSUM [16,512] f32.
Host: bincount(label, weights=w) -> T_c, counts; diag-sum of PSUM -> S_c;
tiny C-length final loss in float64.

bf16 input rationale: memory-regime problem; the kernel's internal
precision choice. Per-element rel error ~2^-9 on x => <=1.1% on exp;
class sums average 4096+ points per core so final loss error ~1e-4,
far inside the 2e-2 gate.
"""

import sys

for _p in ("/opt/trn_rl_repo", "/root/.axon_site/_ro/trn_rl_repo"):
    if _p not in sys.path:
        sys.path.insert(0, _p)

from contextlib import ExitStack

import numpy as np
import ml_dtypes

import concourse.bass as bass
from concourse import mybir, tile

# ---------------- problem constants (hardcoded per contract) ----------------
N_TOTAL = 1048576
C = 32
NCORES = 8
N_CORE = N_TOTAL // NCORES  # 131072
P = 128
PAGES = N_CORE // P         # 1024 pages per partition
GM = 16                     # pages per matmul group == PSUM M dim

# tile sizes in pages: small first (prime ACT early), big middle, small tail.
# All multiples of GM=16 so every matmul hits the identical [16, 512] PSUM
# region (one accumulation group; mismatched sub-regions hang the PE).
# Fewer tiles amortize the ~390ns fixed cost per bf16 ACTIVATE.
SPLITS = [16, 32, 64, 128, 128, 128, 128, 128, 128, 96, 32, 16]
assert sum(SPLITS) == PAGES, sum(SPLITS)
OFFS = [0]
for s in SPLITS:
    OFFS.append(OFFS[-1] + s)
# recip/cast batches: process den->rec->recb in groups of tiles
RBATCH_END = [1, 3, 5, 7, 9, 11]  # inclusive tile index ending each batch
W_SPLIT_TILE = 7                # after this tile's w-mult, DMA w[:, :cum]

GAMMA = 0.3
LOSS_WEIGHT = 1.0
LG2 = 0.6931471805599453

BF16 = ml_dtypes.bfloat16


# ---------------- kernel builder (v3) --------------------------------------
def build_nc_v3():
    f32 = mybir.dt.float32
    bf16 = mybir.dt.bfloat16
    nc = bass.Bass()
    strips = [
        nc.dram_tensor(f"s{t}", [P, 33 * s], bf16, kind="ExternalInput")
        for t, s in enumerate(SPLITS)
    ]
    out_d = nc.dram_tensor("out", [GM, GM * C], f32, kind="ExternalOutput")
    w_d = nc.dram_tensor("wout", [P, PAGES], bf16, kind="ExternalOutput")

    with tile.TileContext(nc) as tc, ExitStack() as ctx:
        pool = ctx.enter_context(tc.tile_pool(name="work", bufs=6))
        tpool = ctx.enter_context(tc.tile_pool(name="tree", bufs=2))
        once = ctx.enter_context(tc.tile_pool(name="once", bufs=1))
        psum = ctx.enter_context(
            tc.tile_pool(name="psum", bufs=1, space=bass.MemorySpace.PSUM)
        )
        ps_s = psum.tile([GM, GM * C], f32)

        den_all = once.tile([P, PAGES], f32)
        rec_all = once.tile([P, PAGES], f32)
        recb_all = once.tile([P, PAGES], bf16)
        w_all = once.tile([P, PAGES], bf16)

        n_mm = sum((s + GM - 1) // GM for s in SPLITS)
        mm_done = 0
        rb = 0  # current recip batch index
        cur = []  # (off, S, ex) of tiles in the open batch
        for t, S in enumerate(SPLITS):
            off = OFFS[t]
            strip = pool.tile([P, 33 * S], bf16, tag="x")
            nc.sync.dma_start(strip[:], strips[t][:])

            ex = pool.tile([P, 33 * S], bf16, tag="e")
            nc.scalar.activation(ex[:], strip[:], mybir.ActivationFunctionType.Exp)
            x3 = ex[:, S:].rearrange("p (s n) -> p s n", n=C)

            # D-reduce as a pairwise tensor_tensor tree: tensor_reduce only
            # has a 1x DVE uop, but bf16 packed tensor_tensor runs 2x.
            src = x3
            width = C
            while width > 2:
                half = width // 2
                nxt = tpool.tile([P, S * half], bf16, tag=f"t{half}")
                n3 = nxt[:].rearrange("p (s n) -> p s n", n=half)
                nc.vector.tensor_tensor(
                    n3,
                    src[:, :, 0:half],
                    src[:, :, half:width],
                    mybir.AluOpType.add,
                )
                src = n3
                width = half
            nc.vector.tensor_reduce(
                den_all[:, off : off + S],
                src,
                axis=mybir.AxisListType.X,
                op=mybir.AluOpType.add,
            )
            cur.append((off, S, ex))

            if t == RBATCH_END[rb]:
                b0 = cur[0][0]
                b1 = OFFS[t + 1]
                nc.vector.reciprocal_approx_fast(
                    rec_all[:, b0:b1], den_all[:, b0:b1]
                )
                with nc.allow_low_precision(reason="bf16 lhsT for PE matmul"):
                    nc.vector.tensor_copy(
                        recb_all[:, b0:b1], rec_all[:, b0:b1]
                    )
                # consumers of recb emitted only after the cast (same-engine
                # queues execute in emission order; cross-engine deps are
                # recorded against emission-time writers)
                for uoff, uS, uex in cur:
                    with nc.allow_low_precision(reason="bf16 w output"):
                        nc.gpsimd.tensor_tensor(
                            w_all[:, uoff : uoff + uS],
                            uex[:, 0:uS],
                            recb_all[:, uoff : uoff + uS],
                            mybir.AluOpType.mult,
                        )
                    for gs in range(0, uS, GM):
                        gl = min(GM, uS - gs)
                        nc.tensor.matmul(
                            ps_s[0:gl, 0 : gl * C],
                            recb_all[:, uoff + gs : uoff + gs + gl],
                            uex[:, uS + gs * C : uS + (gs + gl) * C],
                            start=(mm_done == 0),
                            stop=(mm_done == n_mm - 1),
                        )
                        mm_done += 1
                cur = []
                rb += 1

            if t == W_SPLIT_TILE:
                nc.sync.dma_start(w_d[:, 0 : OFFS[t + 1]], w_all[:, 0 : OFFS[t + 1]])

        nc.sync.dma_start(
            w_d[:, OFFS[W_SPLIT_TILE + 1] :], w_all[:, OFFS[W_SPLIT_TILE + 1] :]
        )
        stage = pool.tile([GM, GM * C], f32, tag="stage")
        nc.scalar.copy(stage[:], ps_s[:])
        nc.sync.dma_start(out_d[:], stage[:])
    return nc


def _finalize_for_hw(nc):
    """Lowerings required by the walrus compile path (not CoreSim)."""
    _split_multi_waits(nc)
    mybir.codegen_inst_isa_subclasses(nc)
    return nc


def _split_multi_waits(nc):
    """Walrus encodes exactly one sync-wait per ISA instruction; Tile can
    attach several. Hoist all-but-the-last wait onto single-wait NoOps
    inserted just before the instruction on the same engine (the sequencer
    executes them in order, so semantics are preserved)."""
    for fn in nc.m.functions:
        for blk in fn.blocks:
            new_list = []
            for ins in blk.instructions:
                si = ins.sync_info
                if si is not None and len(si.on_wait) > 1:
                    waits = list(si.on_wait)
                    for w in waits[:-1]:
                        nop = mybir.InstNoOp(
                            name=f"WS-{nc.next_id()}", ins=[], outs=[]
                        )
                        nop.engine = ins.engine
                        nop.sync_info = mybir.SyncInfo(on_wait=[w], on_update=[])
                        new_list.append(nop)
                    ins.sync_info = mybir.SyncInfo(
                        on_wait=[waits[-1]], on_update=list(si.on_update)
                    )
                new_list.append(ins)
            blk.instructions[:] = new_list


_NC_CACHE = {}


def get_nc():
    if "v3" not in _NC_CACHE:
        _NC_CACHE["v3"] = _finalize_for_hw(build_nc_v3())
    return _NC_CACHE["v3"]


# ---------------- host-side driver ------------------------------------------
def prep_in_maps(cls_score: np.ndarray, label: np.ndarray):
    cls_score = np.ascontiguousarray(cls_score, dtype=np.float32)
    lab = label.astype(np.int64)
    g = cls_score[np.arange(cls_score.shape[0]), lab]
    xb = cls_score.astype(BF16)
    gb = g.astype(BF16)
    in_maps = []
    for k in range(NCORES):
        base = k * N_CORE
        m = {}
        for t, S in enumerate(SPLITS):
            a = base + OFFS[t] * P
            b = a + S * P
            xt = xb[a:b].reshape(P, S * C)
            gt = gb[a:b].reshape(P, S)
            m[f"s{t}"] = np.ascontiguousarray(np.concatenate([gt, xt], axis=1))
        in_maps.append(m)
    return in_maps


def finalize(outs, label: np.ndarray):
    lab = label.astype(np.int64)
    acc = np.zeros((GM, GM * C), dtype=np.float64)
    w_parts = []
    for o in outs:
        acc += o["out"].astype(np.float64)
        w = o["wout"].astype(np.float64)  # [P, PAGES]
        w_parts.append(
            np.concatenate(
                [
                    w[:, OFFS[t] : OFFS[t + 1]].reshape(-1)
                    for t in range(len(SPLITS))
                ]
            )
        )
    blocks = acc.reshape(GM, GM, C)
    s_c = np.zeros(C, dtype=np.float64)
    for mrow in range(GM):
        s_c += blocks[mrow, mrow]

    w_all = np.concatenate(w_parts)
    t_c = np.bincount(lab, weights=w_all, minlength=C)
    counts = np.bincount(lab, minlength=C).astype(np.float64)
    present = counts > 0
    ni = np.log(np.maximum(t_c, 1e-300))
    npr = np.log(np.maximum(s_c, 1e-300))
    log_ngt = np.log(np.maximum(counts, 1.0))
    log_dice = LG2 + ni - np.logaddexp(log_ngt, npr)
    neg_log_dice = np.where(present, -log_dice, 1.0)
    losses = np.where(present, np.power(np.maximum(neg_log_dice, 0.0), GAMMA), 0.0)
    n_present = present.sum()
    return np.float32(LOSS_WEIGHT * losses.sum() / n_present)


def kernel(cls_score: np.ndarray, label: np.ndarray) -> np.ndarray:
    from concourse.bass_utils import run_bass_kernel_spmd

    cls_score = np.asarray(cls_score)
    label = np.asarray(label)
    assert cls_score.shape == (N_TOTAL, C), cls_score.shape
    nc = get_nc()
    in_maps = prep_in_maps(cls_score, label)
    res = run_bass_kernel_spmd(nc, in_maps, core_ids=list(range(NCORES)))
    return finalize(res.results, label)


if __name__ == "__main__":
    rng = np.random.default_rng(0)
    x = rng.standard_normal((N_TOTAL, C), dtype=np.float32)
    lab = rng.integers(0, C, N_TOTAL).astype(np.int32)
    print("loss:", kernel(x, lab))
